# revision 19
# baseline (speedup 1.0000x reference)
"""Causal self-attention on 8 TRN2 NeuronCores (Bass/Tile, SPMD).

Problem: y = CausalSelfAttention(x; Wqkv, bqkv, Wproj, bproj)
  x [B=4, T=2048, C=1024], H=16 heads, D=64.

Sharding: core c = (batch b = c//2, head-half hh = c%2). Each core computes
q/k/v for its 8 heads of its batch (Wqkv column-sharded), full causal
attention for those heads, and a partial output projection (Wproj
row-sharded). Host sums the two partials per batch and adds bproj.

Per-core kernel (all matmuls bf16 with fp32 PSUM accumulation):
  - q,k are produced d-major ([CL, T]) so QK^T needs no transposes;
    scores come out k-major [128 k, 512 q] per tile; the two heads of a
    pair run as concurrent 64x128 PE row-tiles (tile_position 0/64).
  - softmax skips the max-subtraction (scores are O(1) here; exp is safe)
    so it is a single fused exp per k-tile on the Scalar engine; the
    causal mask is a bf16 multiply on the diagonal blocks only. Row sums
    come free from an extra ones-column appended to each per-head V tile
    (M=65 AV matmul), and 1/sum is broadcast across partitions via a
    tiny DRAM round-trip DMA.
  - Sub-diagonal k-tiles are skipped entirely (half the attention work).
  - The attention inner loop is Scalar(exp)-paced, so spare PE slots are
    stuffed with fill units (remaining v tiles, the next head-pair's q/k
    projections) placed at q-block boundaries where the scheduler can
    slide them into the exp-lag bubbles without extra PE tiling-mode
    switches.
  - Softmax normalization runs eagerly per q-block, so the output
    projection (emitted last) becomes schedulable during the final
    head-pair's attention and fills its bubbles.
  - The partial output is written in bf16 (summed in f32 on host).
"""

import math
from contextlib import ExitStack

import numpy as np
import ml_dtypes

import concourse.tile as tile
from concourse import bacc, mybir

BF16 = mybir.dt.bfloat16
F32 = mybir.dt.float32
NPBF16 = ml_dtypes.bfloat16

P = 128  # partitions / k-tile size
QB = 512  # q-block (matmul N; one fp32 PSUM bank)

B, T, C, H, D = 4, 2048, 1024, 16, 64
N_CORES = 8
HL = H // (N_CORES // B)  # heads per core (8)
CL = HL * D  # local head width (512)

# ---------------------------------------------------------------------------
# Per-core Bass program
# ---------------------------------------------------------------------------


def build_kernel(T=T, C=C, HL=HL, D=D, Cout=C):
    CL = HL * D
    n_ct = C // P
    n_mt = CL // P
    n_tt = T // P
    n_qb = T // QB
    n_hp = HL // 2
    dpb = QB // P
    n_cb = Cout // QB
    scale = 1.0 / math.sqrt(D)
    D1 = D + 1
    n_sums = n_hp * n_qb * 2  # one softmax-denominator row per (head, q-block)

    assert C % P == 0 and CL % P == 0 and T % QB == 0 and Cout % QB == 0
    assert HL % 2 == 0 and D == 64 and n_mt == n_hp and n_sums <= P

    nc = bacc.Bacc("TRN2", target_bir_lowering=False, debug=False)
    xT = nc.dram_tensor("xT", [C, T], BF16, kind="ExternalInput")
    wq = nc.dram_tensor("wq", [C, CL], BF16, kind="ExternalInput")
    wk = nc.dram_tensor("wk", [C, CL], BF16, kind="ExternalInput")
    wv = nc.dram_tensor("wv", [C, CL], BF16, kind="ExternalInput")
    wp = nc.dram_tensor("wp", [CL, Cout], BF16, kind="ExternalInput")
    masks = nc.dram_tensor("masks", [P, P], BF16, kind="ExternalInput")
    out = nc.dram_tensor("out", [T, Cout], BF16, kind="ExternalOutput")

    with tile.TileContext(nc) as tc, ExitStack() as ctx:
        persist = ctx.enter_context(tc.tile_pool(name="persist", bufs=1))
        # PSUM budget (8 banks): st2 2 x [128,1024] (4) + yt 2 x [65,512]
        # (2) + u512 2 x [128,512] (2)
        ps_u512 = ctx.enter_context(tc.tile_pool(name="ps_u512", bufs=2, space="PSUM"))
        ps_yt = ctx.enter_context(tc.tile_pool(name="ps_yt", bufs=2, space="PSUM"))
        ps_st2 = ctx.enter_context(tc.tile_pool(name="ps_st2", bufs=2, space="PSUM"))
        ppool = ctx.enter_context(tc.tile_pool(name="ppool", bufs=6))
        spool = ctx.enter_context(tc.tile_pool(name="spool", bufs=4))
        bcpool = ctx.enter_context(tc.tile_pool(name="bcpool", bufs=4))
        stage = ctx.enter_context(tc.tile_pool(name="stage", bufs=4))
        dram = ctx.enter_context(tc.tile_pool(name="dram", bufs=1, space="DRAM"))

        # ---- persistent tiles ----
        xT_sb = [persist.tile([P, T], BF16, tag=f"xT{i}", name=f"xT{i}") for i in range(n_ct)]
        wv_sb = [persist.tile([P, CL], BF16, tag=f"wv{i}", name=f"wv{i}") for i in range(n_ct)]
        wq_sb = [persist.tile([P, CL], BF16, tag=f"wq{i}", name=f"wq{i}") for i in range(n_ct)]
        wk_sb = [persist.tile([P, CL], BF16, tag=f"wk{i}", name=f"wk{i}") for i in range(n_ct)]
        wp_sb = [persist.tile([P, Cout], BF16, tag=f"wp{i}", name=f"wp{i}") for i in range(n_mt)]
        trimask = persist.tile([P, P], BF16, tag="trimask", name="trimask")
        v1_sb = [
            persist.tile([P, HL * D1], BF16, tag=f"v1_{tt}", name=f"v1_{tt}")
            for tt in range(n_tt)
        ]
        yT_sb = [persist.tile([P, T], BF16, tag=f"yT{m}", name=f"yT{m}") for m in range(n_mt)]

        sums_d = dram.tile([n_sums, QB], F32, tag="sums_d", name="sums_d")
        recips_d = dram.tile([n_sums, QB], F32, tag="recips_d", name="recips_d")

        # ---- input DMAs: descriptor generation costs ~630ns serial per
        # dma_start per queue, so spread the issue across four otherwise
        # idle engine queues (xT paces the v matmuls; wv pairs with it)
        for i in range(n_ct):
            nc.sync.dma_start(xT_sb[i][:], xT[i * P : (i + 1) * P, :])
            nc.gpsimd.dma_start(wv_sb[i][:], wv[i * P : (i + 1) * P, :])
        nc.scalar.dma_start(trimask[:], masks[:])
        for i in range(n_ct):
            nc.scalar.dma_start(wk_sb[i][:], wk[i * P : (i + 1) * P, :])
        for i in range(n_ct):
            nc.scalar.dma_start(wq_sb[i][:], wq[i * P : (i + 1) * P, :])
        for m in range(n_mt):
            nc.gpsimd.dma_start(wp_sb[m][:], wp[m * P : (m + 1) * P, :])

        # ---- v: interleaved ones column per head: v1 [T, HL*(D+1)] ----
        def emit_v(tt):
            t = v1_sb[tt]
            ones_view = t[:].rearrange("p (h e) -> p h e", h=HL)[:, :, D : D + 1]
            nc.vector.memset(ones_view, 1.0)
            ps = ps_u512.tile([P, CL], F32, tag="u512", name="u512")
            for c in range(n_ct):
                nc.tensor.matmul(
                    ps[:],
                    xT_sb[c][:, tt * P : (tt + 1) * P],
                    wv_sb[c][:],
                    start=(c == 0),
                    stop=(c == n_ct - 1),
                    skip_group_check=True,
                )
            dst_view = t[:].rearrange("p (h e) -> p h e", h=HL)[:, :, 0:D]
            src_view = ps[:].rearrange("p (h e) -> p h e", h=HL)
            nc.vector.tensor_copy(dst_view, src_view)

        # ---- qk: one d-major [128, QB] projection tile per (hp, b, q/k) ---
        q_d = [[] for _ in range(n_mt)]
        k_d = [[] for _ in range(n_mt)]
        for hp in range(n_hp):
            for b in range(n_qb):
                for name, dst in (("k", k_d), ("q", q_d)):
                    dst[hp].append(
                        persist.tile(
                            [P, QB], BF16, tag=f"{name}d{hp}_{b}", name=f"{name}d{hp}_{b}"
                        )
                    )

        def emit_qk_half(hp, b, which):
            w_sb = wk_sb if which == "k" else wq_sb
            t = (k_d if which == "k" else q_d)[hp][b]
            ps = ps_u512.tile([P, QB], F32, tag="u512", name="u512")
            for c in range(n_ct):
                nc.tensor.matmul(
                    ps[:],
                    w_sb[c][:, hp * P : (hp + 1) * P],
                    xT_sb[c][:, b * QB : (b + 1) * QB],
                    start=(c == 0),
                    stop=(c == n_ct - 1),
                    skip_group_check=True,
                )
            nc.vector.tensor_copy(t[:], ps[:])

        def emit_qk(hp, bs):
            for b in bs:
                emit_qk_half(hp, b, "k")
                emit_qk_half(hp, b, "q")

        # ---- attention for one (head-pair, q-block) ----
        def emit_attn_qb(hp, qb):
            yts = [
                ps_yt.tile([D1, QB], F32, tag="yt", name=f"yt{i}") for i in range(2)
            ]
            n_kt = dpb * qb + dpb

            def emit_av(kt, pt):
                # diagonal k-tiles only touch q-columns >= P*m
                q0 = P * max(kt - dpb * qb, 0)
                for i in range(2):
                    h = 2 * hp + i
                    nc.tensor.matmul(
                        yts[i][:, q0:QB],
                        v1_sb[kt][:, h * D1 : (h + 1) * D1],
                        pt[:, i * QB + q0 : (i + 1) * QB],
                        start=(kt == 0),
                        stop=(kt == n_kt - 1),
                        skip_group_check=True,
                    )

            pending = []
            for kt in range(n_kt):
                m = kt - dpb * qb  # >=0: diagonal tile index
                s0 = P * max(m, 0)  # diagonal tiles: only q-cols >= P*m used
                # combined scores for both heads: [128 k, 1024]
                st = ps_st2.tile([P, 2 * QB], F32, tag="st2", name="st2")
                for i in range(2):
                    base = 64 * i
                    nc.tensor.matmul(
                        st[:, i * QB + s0 : (i + 1) * QB],
                        k_d[hp][kt // dpb][
                            base : base + 64, (kt % dpb) * P : (kt % dpb + 1) * P
                        ],
                        q_d[hp][qb][base : base + 64, s0:],
                        start=True,
                        stop=True,
                    )
                pt = ppool.tile([P, 2 * QB], BF16, tag="pt", name="pt")
                if m <= 0:
                    # one full-width exp covering both heads
                    nc.scalar.activation(
                        pt[:], st[:], mybir.ActivationFunctionType.Exp, scale=scale
                    )
                else:
                    # one strided exp covering both heads' live columns
                    nc.scalar.activation(
                        pt[:].rearrange("p (i q) -> p i q", i=2)[:, :, s0:],
                        st[:].rearrange("p (i q) -> p i q", i=2)[:, :, s0:],
                        mybir.ActivationFunctionType.Exp,
                        scale=scale,
                    )
                if m >= 0:
                    q0 = P * m
                    sl = pt[:].rearrange("p (i q) -> p i q", i=2)[
                        :, :, q0 : q0 + P
                    ]
                    nc.vector.tensor_mul(
                        sl, sl, trimask[:, None, :].broadcast_to([P, 2, P])
                    )
                # stagger: AV lags the scores by 2 k-tiles so the PE
                # never queue-blocks on exp
                pending.append((kt, pt))
                if len(pending) > 2:
                    emit_av(*pending.pop(0))
            for item in pending:
                emit_av(*item)

            # epilogue: one PSUM->SBUF copy per head, then DMA out the
            # unnormalized y (casting SWDGE) and the denominator row
            for i in range(2):
                ys = spool.tile([D1, QB], F32, tag="ys", name="ys")
                nc.vector.tensor_copy(ys[:], yts[i][:])
                nc.gpsimd.dma_start(
                    yT_sb[hp][64 * i : 64 * i + 64, qb * QB : (qb + 1) * QB],
                    ys[0:D, :],
                )
                s = (hp * n_qb + qb) * 2 + i
                nc.sync.dma_start(sums_d[s : s + 1, :], ys[D : D + 1, :])

        def emit_norm_qb(hp, qb):
            # eager per-q-block normalization: reciprocal of the two
            # denominator rows, DRAM round trip to broadcast across
            # partitions, then scale this q-block of yT
            s0 = (hp * n_qb + qb) * 2
            allsums = stage.tile([2, QB], F32, tag="allsums", name="allsums")
            nc.sync.dma_start(allsums[:], sums_d[s0 : s0 + 2, :])
            allrec = stage.tile([2, QB], F32, tag="allrec", name="allrec")
            nc.vector.reciprocal_approx_fast(allrec[:], allsums[:])
            nc.sync.dma_start(recips_d[s0 : s0 + 2, :], allrec[:])
            bc = bcpool.tile([P, QB], F32, tag="bc", name="bc")
            for i in range(2):
                s = s0 + i
                nc.sync.dma_start(
                    bc[64 * i : 64 * i + 64, :],
                    recips_d[s : s + 1, :].to_broadcast((64, QB)),
                )
            sl = yT_sb[hp][:, qb * QB : (qb + 1) * QB]
            nc.vector.tensor_mul(sl, sl, bc[:])

        # ---- schedule ----
        # Upfront (PE-dense; ACT idle anyway): v[0:12], qk(hp0, all).
        # The rest becomes boundary fill: fill units emitted at q-block
        # boundaries slide into the exp-lag bubbles of the neighboring
        # attention phases. Each head-pair's attention carries the first
        # half of the next pair's qk and the tail of its own.
        for tt in range(min(3 * dpb, n_tt)):
            emit_v(tt)
        emit_qk(0, range(min(2, n_qb)))

        # fill plan: fills[(hp, qb)] = list of thunks emitted after
        # emit_attn_qb(hp, qb)
        fills = {(hp, qb): [] for hp in range(n_hp) for qb in range(n_qb)}

        def fqk(hp, b, which):
            return lambda: emit_qk_half(hp, b, which)

        # hp0 qb0/qb1 boundaries: own qk tail + last v tiles
        if n_qb > 2:
            fills[(0, 0)] += [fqk(0, 2, "k"), fqk(0, 2, "q")]
        if n_qb > 3:
            fills[(0, 1)] += [fqk(0, 3, "k"), fqk(0, 3, "q")]
        for tt in range(3 * dpb, n_tt):
            fills[(0, min(1, n_qb - 1))].append(lambda tt=tt: emit_v(tt))
        # each hp carries b0/b1 of the next pair late, each pair does its
        # own b2/b3 early (deadline: before its own qb2/qb3)
        for hp in range(1, n_hp):
            fills[(hp - 1, n_qb - 2)] += [fqk(hp, 0, "k"), fqk(hp, 0, "q")]
            fills[(hp - 1, n_qb - 1)] += [fqk(hp, 1, "k"), fqk(hp, 1, "q")]
            if n_qb > 2:
                fills[(hp, 0)] += [fqk(hp, 2, "k"), fqk(hp, 2, "q")]
            if n_qb > 3:
                fills[(hp, 1)] += [fqk(hp, 3, "k"), fqk(hp, 3, "q")]

        for hp in range(n_hp):
            for qb in range(n_qb):
                emit_attn_qb(hp, qb)
                for thunk in fills[(hp, qb)]:
                    thunk()
                emit_norm_qb(hp, qb)

        # ---- output projection (partial over this core's heads) ----
        # Emitted last; eager norms make the early tt tiles ready during
        # the last head-pair's attention, so the scheduler uses them as
        # fill there.
        for tt in range(n_tt):
            for cb in range(n_cb):
                ps = ps_u512.tile([P, QB], F32, tag="u512", name="u512")
                for m in range(n_mt):
                    nc.tensor.matmul(
                        ps[:],
                        yT_sb[m][:, tt * P : (tt + 1) * P],
                        wp_sb[m][:, cb * QB : (cb + 1) * QB],
                        start=(m == 0),
                        stop=(m == n_mt - 1),
                        skip_group_check=True,
                    )
                ost = stage.tile([P, QB], BF16, tag="ostage", name="ostage")
                nc.vector.tensor_copy(ost[:], ps[:])
                nc.sync.dma_start(
                    out[tt * P : (tt + 1) * P, cb * QB : (cb + 1) * QB], ost[:]
                )

    nc.compile()
    return nc


_PROGRAM_CACHE = {}


def _get_program(C_eff):
    key = C_eff
    if key not in _PROGRAM_CACHE:
        _PROGRAM_CACHE[key] = build_kernel(T=T, C=C_eff, HL=HL, D=D, Cout=C)
    return _PROGRAM_CACHE[key]


def _make_in_maps(x, Wqkv, bqkv):
    """Shard + cast inputs for the 8 cores. Returns (in_maps, C_eff)."""
    if np.any(bqkv):
        # Fold the qkv bias in as an extra contraction row (x gains a ones
        # column), zero-padded up to a multiple of 128.
        C_eff = ((C + 1 + P - 1) // P) * P
        Waug = np.zeros((C_eff, 3 * C), dtype=np.float32)
        Waug[:C] = Wqkv
        Waug[C] = bqkv
    else:
        C_eff = C
        Waug = Wqkv

    masks = (np.arange(P)[:, None] <= np.arange(P)[None, :]).astype(NPBF16)
    in_maps = []
    for core in range(N_CORES):
        b, hh = divmod(core, N_CORES // B)
        xT = np.zeros((C_eff, T), dtype=np.float32)
        xT[:C] = x[b].T
        if C_eff > C:
            xT[C] = 1.0
        c0 = hh * CL
        in_maps.append(
            {
                "xT": xT.astype(NPBF16),
                "wq": np.ascontiguousarray(Waug[:, 0 * C + c0 : 0 * C + c0 + CL]).astype(NPBF16),
                "wk": np.ascontiguousarray(Waug[:, 1 * C + c0 : 1 * C + c0 + CL]).astype(NPBF16),
                "wv": np.ascontiguousarray(Waug[:, 2 * C + c0 : 2 * C + c0 + CL]).astype(NPBF16),
                "wp": None,  # filled below (depends only on hh)
                "masks": masks,
            }
        )
    return in_maps, C_eff


def _run(x, Wqkv, bqkv, Wproj, bproj, trace=False):
    from concourse.bass_utils import run_bass_kernel_spmd

    in_maps, C_eff = _make_in_maps(x, Wqkv, bqkv)
    wp_by_hh = [
        np.ascontiguousarray(Wproj[hh * CL : (hh + 1) * CL, :]).astype(NPBF16)
        for hh in range(N_CORES // B)
    ]
    for core in range(N_CORES):
        in_maps[core]["wp"] = wp_by_hh[core % (N_CORES // B)]

    nc = _get_program(C_eff)
    res = run_bass_kernel_spmd(
        nc, in_maps, core_ids=list(range(N_CORES)), trace=trace
    )

    halves = N_CORES // B
    y = np.empty((B, T, C), dtype=np.float32)
    for b in range(B):
        acc = res.results[b * halves]["out"].astype(np.float32)
        for hh in range(1, halves):
            acc = acc + res.results[b * halves + hh]["out"].astype(np.float32)
        y[b] = acc + bproj.astype(np.float32)
    return y, res


def kernel(x, Wqkv, bqkv, Wproj, bproj):
    y, _ = _run(
        np.asarray(x, dtype=np.float32),
        np.asarray(Wqkv, dtype=np.float32),
        np.asarray(bqkv, dtype=np.float32),
        np.asarray(Wproj, dtype=np.float32),
        np.asarray(bproj, dtype=np.float32),
        trace=False,
    )
    return y
